# revision 10
# baseline (speedup 1.0000x reference)
"""BEiT-style attention (B=16, N=577, C=768, H=12) on 8 TRN2 NeuronCores.

Strategy: pure data-parallel over batch (2 batches/core, no collectives).
Per-core kernel computes attention in a transposed-score layout (S^T with
softmax axis on partitions) which needs zero on-device transposes:

  qT,kT  [d, n] = Wqk8 @ x8^T        fp8 DoubleRow hi/lo (4.5 units vs 6)
  q8,k8  [32, 2, n]                  DMA partition-fold, 4 heads/tile
  S^T    [m, n] = k8.T-slices @ q8   fp8 DoubleRow (0.5 cycles/row)
  expS^T [m, n] = exp(sc*S^T) * exp(relposT)   (scale folded into ACT)
  outT   [d, n] = [v | 1].T @ expS^T (row 64 = softmax denominator; bf16)
  out    [n,co] = outT_norm.T-slices @ Wp^T + bias  (bf16 out, host upcast)

fp8 trick: weights pre-scaled by 32 on host (std 0.02 is fp8-denormal
territory); 1/32^2 folded into the exp scale, 1/32 into proj weights.
x and W are split hi/lo fp8 so DoubleRow matmuls carry ~2^-8 relative
error (better than bf16) at 0.5 cycles/row.
"""

import os
import sys
from contextlib import ExitStack

import numpy as np

sys.path.insert(0, "/opt/trn_rl_repo")

# the kernel executes through jax/PJRT on the axon-tunneled NeuronCores; a
# JAX_PLATFORMS=cpu pin (useful for pure-reference runs) would hide them
if os.environ.get("JAX_PLATFORMS", "") == "cpu":
    os.environ.pop("JAX_PLATFORMS", None)

import ml_dtypes  # noqa: E402

from concourse import bacc, mybir  # noqa: E402
import concourse.bass as bass  # noqa: E402
import concourse.tile as tile  # noqa: E402
from concourse.bass_utils import run_bass_kernel_spmd  # noqa: E402

BF16 = mybir.dt.bfloat16
F32 = mybir.dt.float32
FP8 = mybir.dt.float8e4
NPBF16 = ml_dtypes.bfloat16
NPFP8 = ml_dtypes.float8_e4m3
AF = mybir.ActivationFunctionType
PM = mybir.MatmulPerfMode

B, N, C = 16, 577, 768
H, HD = 12, 64
NCORES = 8
BL = B // NCORES  # local batches per core
KC = C // 128  # contraction tiles over channels
SCALE = HD ** -0.5
WS = 32.0  # host weight pre-scale (exact power of 2)
EXP_SCALE = SCALE / (WS * WS)

# token-dim partition tiles (offset, width)
NP = 592  # padded token-dim slot: DoubleRow pair strides must be %16==0
NT = [(0, 128), (128, 128), (256, 128), (384, 128), (512, 65)]
# token-dim free chunks (halves of 577, each fits one PSUM bank / <=512 mm)
FC = [(0, 289), (289, 288)]
# chunks aligned to PSUM bank boundary for fused two-bank [.,577] psum tiles
FB = [(0, 512), (512, 65)]
# channel free chunks for 768-wide outputs
CC = [(0, 384), (384, 384)]


def build_graph():
    nc = bacc.Bacc("TRN2", target_bir_lowering=False, debug=False, num_devices=NCORES)

    # x8: [BL, 128, 2(hl: lo,hi), KC, N]; w8: [128, 2(hl: hi,lo), KC, chan]
    x8_d = nc.dram_tensor("x8", (BL, 128, 2, KC, NP), FP8, kind="ExternalInput").ap()
    wqk_d = nc.dram_tensor("wqk8", (128, 2, KC, 2 * C), FP8, kind="ExternalInput").ap()
    wv_d = nc.dram_tensor("wv8", (128, 2, KC, C), FP8, kind="ExternalInput").ap()
    pw_d = nc.dram_tensor("pwT", (C, C), BF16, kind="ExternalInput").ap()
    eb_d = nc.dram_tensor("ebT", (H, 640, N), BF16, kind="ExternalInput").ap()
    qkb_d = nc.dram_tensor("qkb", (128, 2 * KC), F32, kind="ExternalInput").ap()
    pbc_d = nc.dram_tensor("pbc", (128, KC), F32, kind="ExternalInput").ap()
    out_d = nc.dram_tensor("out", (BL, C, N), BF16, kind="ExternalOutput").ap()

    with tile.TileContext(nc) as tc, ExitStack() as ctx:
        res = ctx.enter_context(tc.tile_pool(name="res", bufs=1))
        ebp = ctx.enter_context(tc.tile_pool(name="ebp", bufs=4))
        e1p = ctx.enter_context(tc.tile_pool(name="e1p", bufs=6))
        estp = ctx.enter_context(tc.tile_pool(name="estp", bufs=8))
        rowp = ctx.enter_context(tc.tile_pool(name="rowp", bufs=6))
        bcp = ctx.enter_context(tc.tile_pool(name="bcp", bufs=6))
        finp = ctx.enter_context(tc.tile_pool(name="finp", bufs=4))
        ps_mm = ctx.enter_context(
            tc.tile_pool(name="ps_mm", bufs=2, space=bass.MemorySpace.PSUM)
        )
        ps_st = ctx.enter_context(
            tc.tile_pool(name="ps_st", bufs=2, space=bass.MemorySpace.PSUM)
        )
        ps_pv = ctx.enter_context(
            tc.tile_pool(name="ps_pv", bufs=2, space=bass.MemorySpace.PSUM)
        )

        # ---- resident tiles ----
        wqk = res.tile([128, 2 * KC * 2 * C], FP8, name="wqk", tag="wqk")
        wv = res.tile([128, 2 * KC * C], FP8, name="wv", tag="wv")
        pw = res.tile([128, KC * C], BF16, name="pw", tag="pw")
        xt = [res.tile([128, 2 * KC * NP], FP8, name=f"xt{b}", tag=f"xt{b}") for b in range(BL)]
        # pre-shuffle q/k (fp8, d on partitions): col t*N.. = chan block t
        qt8 = [res.tile([128, KC * N], FP8, name=f"qt8{b}", tag=f"qt8{b}") for b in range(BL)]
        kt8 = [res.tile([128, KC * N], FP8, name=f"kt8{b}", tag=f"kt8{b}") for b in range(BL)]
        # post-shuffle packed (4 heads x 32 partitions, halves in free dim)
        qs8 = [res.tile([128, 3 * 2 * NP], FP8, name=f"qs8{b}", tag=f"qs8{b}") for b in range(BL)]
        ks8 = [res.tile([128, 3 * 2 * NP], FP8, name=f"ks8{b}", tag=f"ks8{b}") for b in range(BL)]
        vt = [
            [res.tile([128, H * (HD + 1)], BF16, name=f"vt{b}_{m}", tag=f"vt{b}_{m}") for m in range(len(NT))]
            for b in range(BL)
        ]
        ot = [
            [res.tile([128, N], BF16, name=f"ot{b}_{k}", tag=f"ot{b}_{k}") for k in range(KC)]
            for b in range(BL)
        ]
        qkb = res.tile([128, 2 * KC], F32, name="qkb_s", tag="qkb_s")
        pbc = res.tile([128, KC], F32, name="pbc_s", tag="pbc_s")

        def x8v(b):  # [128, hl, kc, NP]
            return xt[b][:].rearrange("p (a k n) -> p a k n", a=2, k=KC)

        def wqk8v():  # [128, hl, kc, 2C]
            return wqk[:].rearrange("p (a k c) -> p a k c", a=2, k=KC)

        def wv8v():  # [128, hl, kc, C]
            return wv[:].rearrange("p (a k c) -> p a k c", a=2, k=KC)

        # ---- input DMA: q/k weights + x(hi) first (gate the first matmuls) ----
        # x8 layout hl: 0=lo 1=hi ; w8 layout hl: 0=hi 1=lo
        def dma_x(b, hl, k0, kn, eng=None):
            (eng or nc.sync).dma_start(
                x8v(b)[:, hl, k0 : k0 + kn, :],
                x8_d[b, :, hl, k0 : k0 + kn, :],
            )

        def dma_w(dst_v, src_d, hl, k0, kn, eng=None):
            (eng or nc.sync).dma_start(
                dst_v[:, hl, k0 : k0 + kn, :],
                src_d[:, hl, k0 : k0 + kn, :],
            )

        dma_w(wqk8v(), wqk_d, 0, 0, 2)
        dma_x(0, 1, 0, 2)
        dma_w(wqk8v(), wqk_d, 1, 0, 2)
        dma_x(0, 0, 0, 2)
        nc.sync.dma_start(qkb[:], qkb_d[:])
        for k0 in (2, 4):
            dma_w(wqk8v(), wqk_d, 0, k0, 2)
            dma_x(0, 1, k0, 2)
            dma_w(wqk8v(), wqk_d, 1, k0, 2)
            dma_x(0, 0, k0, 2)
        for hl in (0, 1):
            dma_w(wv8v(), wv_d, hl, 0, KC)
        for hl in (1, 0):
            dma_x(1, hl, 0, KC)

        # ones column (index HD) interleaved per head in the v tiles
        for b in range(BL):
            for m in range(len(NT)):
                vints = vt[b][m][:].rearrange("p (h e) -> p h e", h=H)
                nc.vector.memset(vints[:, :, HD : HD + 1], 1.0)

        def dr_matmuls(ps_slice, lhs_hi_pair, lhs_cross, rhs_hi_pair, rhs_cross):
            """9 DoubleRow matmuls: 3 hi-hi (kc pairs) + 6 cross (per kc)."""
            first = True
            for j in range(KC // 2):
                nc.tensor.matmul(
                    ps_slice, lhs_hi_pair(j), rhs_hi_pair(j),
                    start=first, stop=False, perf_mode=PM.DoubleRow,
                )
                first = False
            for k in range(KC):
                nc.tensor.matmul(
                    ps_slice, lhs_cross(k), rhs_cross(k),
                    start=False, stop=(k == KC - 1), perf_mode=PM.DoubleRow,
                )

        def emit_qkv_v(b, m):
            m0, mw = NT[m]
            xv = x8v(b)
            wvv = wv8v()
            for ci, (c0, cw) in enumerate(CC):
                ps = ps_mm.tile([128, 512], F32, name=f"psv{b}_{m}_{ci}", tag="mm")
                dr_matmuls(
                    ps[:mw, :cw],
                    lambda j: xv[:, 1, 2 * j : 2 * j + 2, m0 : m0 + mw],
                    lambda k: xv[:, :, k, m0 : m0 + mw],
                    lambda j: wvv[:, 0, 2 * j : 2 * j + 2, c0 : c0 + cw],
                    lambda k: wvv[:, :, k, c0 : c0 + cw],
                )
                nh = cw // HD
                dst = vt[b][m][:mw, ci * nh * (HD + 1) : (ci + 1) * nh * (HD + 1)]
                dst = dst.rearrange("p (h e) -> p h e", h=nh)[:, :, 0:HD]
                src = ps[:mw, :cw].rearrange("p (h e) -> p h e", h=nh)
                nc.vector.tensor_copy(dst, src)

        def emit_qkv_qk(b, ct, ts=None):
            xv = x8v(b)
            wqv = wqk8v()
            for t in ts if ts is not None else (ct, KC + ct):
                for n0, nw in FC:
                    ps = ps_mm.tile([128, 512], F32, name=f"psqk{b}_{t}_{n0}", tag="mm")
                    dr_matmuls(
                        ps[:, :nw],
                        lambda j: wqv[:, 0, 2 * j : 2 * j + 2, t * 128 : (t + 1) * 128],
                        lambda k: wqv[:, :, k, t * 128 : (t + 1) * 128],
                        lambda j: xv[:, 1, 2 * j : 2 * j + 2, n0 : n0 + nw],
                        lambda k: xv[:, :, k, n0 : n0 + nw],
                    )
                    # psum -> fp8 q/k drain; q (+bias) on ACT, k (plain copy)
                    # on DVE to spread the drain across engines
                    dst = (qt8 if t < KC else kt8)[b]
                    tc_ = t if t < KC else t - KC
                    dslice = dst[:, tc_ * N + n0 : tc_ * N + n0 + nw]
                    with nc.allow_low_precision("fp8 q/k for DoubleRow scores"):
                        if t < KC:
                            nc.scalar.activation(
                                dslice, ps[:, :nw], AF.Identity,
                                bias=qkb[:, t : t + 1],
                            )
                        else:
                            nc.vector.tensor_copy(dslice, ps[:, :nw])

        def emit_shuffle(b, g0, ng):
            """DMA partition-fold for head groups g0..g0+ng-1:
            qs8[64*jt+32*jo+p, g*2*NP+i*NP+n] = qt8[64*jo+32*i+p, (2g+jt)*N+n].
            One DMA per (side, jt, i, jo), g as a strided middle dim; every AP
            is a single-partition-dim 3-dim pattern.
            """
            for srcb, dstb in ((qt8[b], qs8[b]), (kt8[b], ks8[b])):
                for jt in range(2):
                    for i in range(2):
                        for jo in range(2):
                            sv = srcb[64 * jo + 32 * i : 64 * jo + 32 * i + 32, :].rearrange(
                                "p (t2 two n) -> p t2 two n", two=2, n=N
                            )[:, g0 : g0 + ng, jt, :]
                            dv = dstb[64 * jt + 32 * jo : 64 * jt + 32 * jo + 32, :].rearrange(
                                "p (t2 two n) -> p t2 two n", two=2, n=NP
                            )[:, g0 : g0 + ng, i, 0:N]
                            nc.sync.dma_start(dv, sv)

        def emit_eb(h, tagsfx):
            eb = ebp.tile([128, 5 * N], BF16, name=f"eb{h}{tagsfx}", tag="eb")
            nc.sync.dma_start(
                eb[:].rearrange("p (m n) -> p m n", m=5),
                eb_d[h].rearrange("(m p) n -> p m n", p=128),
            )
            return eb

        def emit_att_st(h, b, eb):
            """S^T DR matmuls + exp + bias-mult for one head."""
            g, j = h // 4, h % 4
            qv = qs8[b][32 * j : 32 * j + 32, g * 2 * NP : (g + 1) * 2 * NP].rearrange(
                "p (i n) -> p i n", i=2
            )
            kv = ks8[b][32 * j : 32 * j + 32, g * 2 * NP : (g + 1) * 2 * NP].rearrange(
                "p (i n) -> p i n", i=2
            )
            est = []   # per-pair tiles [128, 2N]; est slice for mt m = pair[m//2][:, (m%2)*N:]
            pe1 = pcur = None
            for m, (m0, mw) in enumerate(NT):
                if m % 2 == 0:
                    w = 2 * N if m + 1 < len(NT) else N
                    pcur = estp.tile([128, w], BF16, name=f"est{h}_{b}_{m}", tag="est")
                    pe1 = e1p.tile([128, w], BF16, name=f"e1{h}_{b}_{m}", tag="e1")
                    est.append(pcur)
                co = (m % 2) * N
                ps = ps_st.tile([128, N], F32, name=f"pst{h}_{b}_{m}", tag="st")
                for n0, nw in FB:
                    nc.tensor.matmul(
                        ps[:mw, n0 : n0 + nw],
                        kv[:, :, m0 : m0 + mw],
                        qv[:, :, n0 : n0 + nw],
                        start=True,
                        stop=True,
                        perf_mode=PM.DoubleRow,
                        tile_position=(32 * j, 0),
                    )
                nc.scalar.activation(pe1[:mw, co : co + N], ps[:mw, :], AF.Exp, scale=EXP_SCALE)
                if m % 2 == 1 or m == len(NT) - 1:
                    pw_ = co + N
                    # eb multiply: all-SBUF bf16, so Pool can help; route odd
                    # heads' pair tiles to Pool to offload the busy DVE
                    eng = nc.gpsimd if (h % 2 == 1 and m % 2 == 1) else nc.vector
                    eng.tensor_mul(
                        pcur[:mw, 0:pw_],
                        pe1[:mw, 0:pw_],
                        eb[:mw, (m - pw_ // N + 1) * N : (m + 1) * N],
                    )
            return est

        def emit_att_pv(h, b, est):
            ctq = h // 2
            off = (h % 2) * HD
            rr = rowp.tile([1, N], BF16, name=f"rr{h}_{b}", tag="rr")
            bc = bcp.tile([HD, N], BF16, name=f"bc{h}_{b}", tag="bc")
            pvs = []
            for fi, (n0, nw) in enumerate(FB):
                pv = ps_pv.tile([HD + 1, 512], F32, name=f"pv{h}_{b}_{fi}", tag="pv")
                pvs.append(pv)
                for m, (m0, mw) in enumerate(NT):
                    sl = (m % 2) * N + n0
                    nc.tensor.matmul(
                        pv[: HD + 1, :nw],
                        vt[b][m][:mw, h * (HD + 1) : (h + 1) * (HD + 1)],
                        est[m // 2][:mw, sl : sl + nw],
                        start=(m == 0),
                        stop=(m == len(NT) - 1),
                    )
                with nc.allow_low_precision("softmax denominator recip in bf16"):
                    nc.vector.reciprocal(rr[0:1, n0 : n0 + nw], pv[HD : HD + 1, :nw])
            nc.gpsimd.partition_broadcast(bc[:, :], rr[0:1, :], channels=HD)
            for fi, (n0, nw) in enumerate(FB):
                nc.vector.tensor_mul(
                    ot[b][ctq][off : off + HD, n0 : n0 + nw],
                    pvs[fi][0:HD, :nw],
                    bc[:, n0 : n0 + nw],
                )

        def emit_att(h, b, eb):
            emit_att_pv(h, b, emit_att_st(h, b, eb))

        def emit_proj(b, cot):
            """fin^T[co, n] = pw-block.T-slices @ ot; bias is per-partition
            here; host transposes the (C, N) output back to (N, C)."""
            fin = finp.tile([128, N], BF16, name=f"fin{b}_{cot}", tag="fin")
            for n0, nw in FB:
                ps = ps_mm.tile([128, 512], F32, name=f"psp{b}_{cot}_{n0}", tag="mm")
                for k in range(KC):
                    nc.tensor.matmul(
                        ps[:, :nw],
                        pw[:, k * C + cot * 128 : k * C + (cot + 1) * 128],
                        ot[b][k][:, n0 : n0 + nw],
                        start=(k == 0),
                        stop=(k == KC - 1),
                    )
                # alternate ACT/DVE so neither serializes the tail
                if cot % 2 == 0:
                    nc.scalar.activation(
                        fin[:, n0 : n0 + nw],
                        ps[:, :nw],
                        AF.Identity,
                        bias=pbc[:, cot : cot + 1],
                    )
                else:
                    nc.vector.tensor_scalar_add(
                        fin[:, n0 : n0 + nw], ps[:, :nw], pbc[:, cot : cot + 1]
                    )
            nc.sync.dma_start(out_d[b, cot * 128 : (cot + 1) * 128, :], fin[:, :])

        # ---- emission schedule: fill PE during ACT/DVE-bound attention ----
        # pass 1: b0 qk first (shuffle gates attention), v + b1 fillers
        emit_qkv_qk(0, 0)
        emit_qkv_qk(0, 1)
        emit_shuffle(0, 0, 1)
        emit_qkv_v(0, 0)
        for h in range(H):
            eb = emit_eb(h, "a")
            if h < 4:
                emit_qkv_v(0, h + 1)
                emit_qkv_qk(0, h + 2)
                if h == 3:
                    emit_shuffle(0, 1, 2)
            elif h < 9:
                emit_qkv_v(1, h - 4)
                if h == 7:
                    # proj weights aren't consumed until pass 2; issue them
                    # here so they never queue ahead of eb tiles on SP
                    nc.sync.dma_start(
                        pw[:].rearrange("p (k c) -> p k c", k=KC),
                        pw_d[:].rearrange("(k p) c -> p k c", p=128),
                    )
                    nc.sync.dma_start(pbc[:], pbc_d[:])
            else:
                emit_qkv_qk(1, h - 9)
            emit_att(h, 0, eb)
        # pass 2: b1 attention; fillers: remaining b1 qk early, b0 proj late
        emit_qkv_qk(1, 3)
        emit_qkv_qk(1, 4)
        emit_qkv_qk(1, 5)
        emit_shuffle(1, 0, 3)
        emit_proj(0, 0)
        for h in range(H):
            eb = emit_eb(h, "b")
            if h % 2 == 1 and h <= 9:
                emit_proj(0, (h + 1) // 2)
            emit_att(h, 1, eb)
        for cot in range(KC):
            emit_proj(1, cot)

    nc.compile()
    return nc


_NC = None


def get_compiled():
    global _NC
    if _NC is None:
        _NC = build_graph()
    return _NC


def fp8_hilo(a):
    """Split fp32 array into (lo, hi) fp8e4m3 with hi + lo ~= a."""
    hi = a.astype(NPFP8)
    lo = (a - hi.astype(np.float32)).astype(NPFP8)
    return lo, hi


def prep_in_maps(x, rel_pos_bias, qkv_weight, q_bias, v_bias, proj_weight, proj_bias):
    x = np.asarray(x, np.float32)
    rel_pos_bias = np.asarray(rel_pos_bias, np.float32)
    qkv_weight = np.asarray(qkv_weight, np.float32)
    q_bias = np.asarray(q_bias, np.float32)
    v_bias = np.asarray(v_bias, np.float32)
    proj_weight = np.asarray(proj_weight, np.float32)
    proj_bias = np.asarray(proj_bias, np.float32)

    # x8: (B, 128, 2(lo,hi), KC, N)
    xT = np.ascontiguousarray(x.transpose(0, 2, 1))  # (B, C, N)
    xk = xT.reshape(B, KC, 128, N).transpose(0, 2, 1, 3)  # (B, 128, KC, N)
    x_lo, x_hi = fp8_hilo(xk)
    x8 = np.zeros((B, 128, 2, KC, NP), NPFP8)
    x8[..., :N] = np.stack([x_lo, x_hi], axis=2)

    # w8: (128, 2(hi,lo), KC, chan), pre-scaled by WS
    def w8_of(wT, chan):  # wT (C, chan)
        wk = (WS * wT).reshape(KC, 128, chan).transpose(1, 0, 2)  # (128, KC, chan)
        lo, hi = fp8_hilo(wk)
        return np.ascontiguousarray(np.stack([hi, lo], axis=1))  # (128,2,KC,chan)

    wqk8 = w8_of(qkv_weight[: 2 * C].T, 2 * C)
    wv8 = w8_of(qkv_weight[2 * C :].T, C)
    pwT = np.ascontiguousarray(proj_weight.T / WS).astype(NPBF16)  # (C, C)
    ebT = np.zeros((H, 640, N), NPBF16)
    ebT[:, :N] = np.exp(rel_pos_bias.transpose(0, 2, 1).astype(np.float64)).astype(NPBF16)

    qkb = np.zeros((128, 2 * KC), np.float32)
    for t in range(KC):
        qkb[:, t] = WS * q_bias[t * 128 : (t + 1) * 128]

    pbe = (proj_bias + v_bias @ proj_weight.T).astype(np.float32)  # (C,)
    pbc = np.ascontiguousarray(pbe.reshape(KC, 128).T)  # [p, cot] = pbe[cot*128+p]

    shared = {"wqk8": wqk8, "wv8": wv8, "pwT": pwT, "ebT": ebT, "qkb": qkb, "pbc": pbc}
    in_maps = []
    for i in range(NCORES):
        m = dict(shared)
        m["x8"] = np.ascontiguousarray(x8[i * BL : (i + 1) * BL])
        in_maps.append(m)
    return in_maps


def run(inputs, trace=False, **kw):
    nc = get_compiled()
    in_maps = prep_in_maps(**inputs)
    res = run_bass_kernel_spmd(nc, in_maps, core_ids=list(range(NCORES)), trace=trace, **kw)
    outT = np.concatenate([np.asarray(r["out"]) for r in res.results], axis=0)  # (B, C, N)
    out = np.ascontiguousarray(outT.transpose(0, 2, 1)).astype(np.float32)
    return out, res


def kernel(**inputs):
    out, _ = run(inputs, trace=False)
    return out


# revision 13
# speedup vs baseline: 1.0097x; 1.0097x over previous
"""BEiT-style attention (B=16, N=577, C=768, H=12) on 8 TRN2 NeuronCores.

Strategy: pure data-parallel over batch (2 batches/core, no collectives).
Per-core kernel computes attention in a transposed-score layout (S^T with
softmax axis on partitions) which needs zero on-device transposes:

  qT,kT  [d, n] = Wqk8 @ x8^T        fp8 DoubleRow hi/lo (4.5 units vs 6)
  q8,k8  [32, 2, n]                  DMA partition-fold, 4 heads/tile
  S^T    [m, n] = k8.T-slices @ q8   fp8 DoubleRow (0.5 cycles/row)
  expS^T [m, n] = exp(sc*S^T) * exp(relposT)   (scale folded into ACT)
  outT   [d, n] = [v | 1].T @ expS^T (row 64 = softmax denominator; bf16)
  out    [n,co] = outT_norm.T-slices @ Wp^T + bias  (bf16 out, host upcast)

fp8 trick: weights pre-scaled by 32 on host (std 0.02 is fp8-denormal
territory); 1/32^2 folded into the exp scale, 1/32 into proj weights.
x and W are split hi/lo fp8 so DoubleRow matmuls carry ~2^-8 relative
error (better than bf16) at 0.5 cycles/row.
"""

import os
import sys
from contextlib import ExitStack

import numpy as np

sys.path.insert(0, "/opt/trn_rl_repo")

# the kernel executes through jax/PJRT on the axon-tunneled NeuronCores; a
# JAX_PLATFORMS=cpu pin (useful for pure-reference runs) would hide them
if os.environ.get("JAX_PLATFORMS", "") == "cpu":
    os.environ.pop("JAX_PLATFORMS", None)

import ml_dtypes  # noqa: E402

from concourse import bacc, mybir  # noqa: E402
import concourse.bass as bass  # noqa: E402
import concourse.tile as tile  # noqa: E402
from concourse.bass_utils import run_bass_kernel_spmd  # noqa: E402

BF16 = mybir.dt.bfloat16
F32 = mybir.dt.float32
FP8 = mybir.dt.float8e4
NPBF16 = ml_dtypes.bfloat16
NPFP8 = ml_dtypes.float8_e4m3
AF = mybir.ActivationFunctionType
PM = mybir.MatmulPerfMode

B, N, C = 16, 577, 768
H, HD = 12, 64
NCORES = 8
BL = B // NCORES  # local batches per core
KC = C // 128  # contraction tiles over channels
SCALE = HD ** -0.5
WS = 32.0  # host weight pre-scale (exact power of 2)
EXP_SCALE = SCALE / (WS * WS)

# token-dim partition tiles (offset, width)
NP = 592  # padded token-dim slot: DoubleRow pair strides must be %16==0
NT = [(0, 128), (128, 128), (256, 128), (384, 128), (512, 65)]
# token-dim free chunks (halves of 577, each fits one PSUM bank / <=512 mm)
FC = [(0, 289), (289, 288)]
# chunks aligned to PSUM bank boundary for fused two-bank [.,577] psum tiles
FB = [(0, 512), (512, 65)]
# channel free chunks for 768-wide outputs
CC = [(0, 384), (384, 384)]


def build_graph():
    nc = bacc.Bacc("TRN2", target_bir_lowering=False, debug=False, num_devices=NCORES)

    # x8: [BL, 128, 2(hl: lo,hi), KC, N]; w8: [128, 2(hl: hi,lo), KC, chan]
    x8_d = nc.dram_tensor("x8", (BL, 128, 2, KC, NP), FP8, kind="ExternalInput").ap()
    wqk_d = nc.dram_tensor("wqk8", (128, 2, KC, 2 * C), FP8, kind="ExternalInput").ap()
    wv_d = nc.dram_tensor("wv8", (128, 2, KC, C), FP8, kind="ExternalInput").ap()
    pw_d = nc.dram_tensor("pwT", (C, C), BF16, kind="ExternalInput").ap()
    eb_d = nc.dram_tensor("ebT", (H, 640, N), BF16, kind="ExternalInput").ap()
    qkb_d = nc.dram_tensor("qkb", (128, 2 * KC), F32, kind="ExternalInput").ap()
    pbc_d = nc.dram_tensor("pbc", (128, KC), F32, kind="ExternalInput").ap()
    out_d = nc.dram_tensor("out", (BL, C, N), BF16, kind="ExternalOutput").ap()

    with tile.TileContext(nc) as tc, ExitStack() as ctx:
        res = ctx.enter_context(tc.tile_pool(name="res", bufs=1))
        ebp = ctx.enter_context(tc.tile_pool(name="ebp", bufs=4))
        e1p = ctx.enter_context(tc.tile_pool(name="e1p", bufs=6))
        estp = ctx.enter_context(tc.tile_pool(name="estp", bufs=8))
        rowp = ctx.enter_context(tc.tile_pool(name="rowp", bufs=6))
        bcp = ctx.enter_context(tc.tile_pool(name="bcp", bufs=6))
        finp = ctx.enter_context(tc.tile_pool(name="finp", bufs=4))
        ps_mm = ctx.enter_context(
            tc.tile_pool(name="ps_mm", bufs=2, space=bass.MemorySpace.PSUM)
        )
        ps_st = ctx.enter_context(
            tc.tile_pool(name="ps_st", bufs=2, space=bass.MemorySpace.PSUM)
        )
        ps_pv = ctx.enter_context(
            tc.tile_pool(name="ps_pv", bufs=2, space=bass.MemorySpace.PSUM)
        )

        # ---- resident tiles ----
        wqk = res.tile([128, 2 * KC * 2 * C], FP8, name="wqk", tag="wqk")
        wv = res.tile([128, 2 * KC * C], FP8, name="wv", tag="wv")
        pw = res.tile([128, KC * C], BF16, name="pw", tag="pw")
        xt = [res.tile([128, 2 * KC * NP], FP8, name=f"xt{b}", tag=f"xt{b}") for b in range(BL)]
        # pre-shuffle q/k (fp8, d on partitions): col t*N.. = chan block t
        qt8 = [res.tile([128, KC * N], FP8, name=f"qt8{b}", tag=f"qt8{b}") for b in range(BL)]
        kt8 = [res.tile([128, KC * N], FP8, name=f"kt8{b}", tag=f"kt8{b}") for b in range(BL)]
        # post-shuffle packed (4 heads x 32 partitions, halves in free dim)
        qs8 = [res.tile([128, 3 * 2 * NP], FP8, name=f"qs8{b}", tag=f"qs8{b}") for b in range(BL)]
        ks8 = [res.tile([128, 3 * 2 * NP], FP8, name=f"ks8{b}", tag=f"ks8{b}") for b in range(BL)]
        vt = [
            [res.tile([128, H * (HD + 1)], BF16, name=f"vt{b}_{m}", tag=f"vt{b}_{m}") for m in range(len(NT))]
            for b in range(BL)
        ]
        ot = [
            [res.tile([128, N], BF16, name=f"ot{b}_{k}", tag=f"ot{b}_{k}") for k in range(KC)]
            for b in range(BL)
        ]
        qkb = res.tile([128, 2 * KC], F32, name="qkb_s", tag="qkb_s")
        pbc = res.tile([128, KC], F32, name="pbc_s", tag="pbc_s")

        def x8v(b):  # [128, hl, kc, NP]
            return xt[b][:].rearrange("p (a k n) -> p a k n", a=2, k=KC)

        def wqk8v():  # [128, hl, kc, 2C]
            return wqk[:].rearrange("p (a k c) -> p a k c", a=2, k=KC)

        def wv8v():  # [128, hl, kc, C]
            return wv[:].rearrange("p (a k c) -> p a k c", a=2, k=KC)

        # ---- input DMA: q/k weights + x(hi) first (gate the first matmuls) ----
        # x8 layout hl: 0=lo 1=hi ; w8 layout hl: 0=hi 1=lo
        def dma_x(b, hl, k0, kn, eng=None):
            (eng or nc.sync).dma_start(
                x8v(b)[:, hl, k0 : k0 + kn, :],
                x8_d[b, :, hl, k0 : k0 + kn, :],
            )

        def dma_w(dst_v, src_d, hl, k0, kn, eng=None):
            (eng or nc.sync).dma_start(
                dst_v[:, hl, k0 : k0 + kn, :],
                src_d[:, hl, k0 : k0 + kn, :],
            )

        dma_w(wqk8v(), wqk_d, 0, 0, 2)
        dma_x(0, 1, 0, 2)
        dma_w(wqk8v(), wqk_d, 1, 0, 2)
        dma_x(0, 0, 0, 2)
        nc.sync.dma_start(qkb[:], qkb_d[:])
        for k0 in (2, 4):
            dma_w(wqk8v(), wqk_d, 0, k0, 2)
            dma_x(0, 1, k0, 2)
            dma_w(wqk8v(), wqk_d, 1, k0, 2)
            dma_x(0, 0, k0, 2)
        for hl in (0, 1):
            dma_w(wv8v(), wv_d, hl, 0, KC)
        for hl in (1, 0):
            dma_x(1, hl, 0, KC)

        # ones column (index HD) interleaved per head in the v tiles
        for b in range(BL):
            for m in range(len(NT)):
                vints = vt[b][m][:].rearrange("p (h e) -> p h e", h=H)
                nc.vector.memset(vints[:, :, HD : HD + 1], 1.0)

        def dr_matmuls(ps_slice, lhs_hi_pair, lhs_cross, rhs_hi_pair, rhs_cross):
            """9 DoubleRow matmuls: 3 hi-hi (kc pairs) + 6 cross (per kc)."""
            first = True
            for j in range(KC // 2):
                nc.tensor.matmul(
                    ps_slice, lhs_hi_pair(j), rhs_hi_pair(j),
                    start=first, stop=False, perf_mode=PM.DoubleRow,
                )
                first = False
            for k in range(KC):
                nc.tensor.matmul(
                    ps_slice, lhs_cross(k), rhs_cross(k),
                    start=False, stop=(k == KC - 1), perf_mode=PM.DoubleRow,
                )

        def emit_qkv_v(b, m):
            m0, mw = NT[m]
            xv = x8v(b)
            wvv = wv8v()
            for ci, (c0, cw) in enumerate(CC):
                ps = ps_mm.tile([128, 512], F32, name=f"psv{b}_{m}_{ci}", tag="mm")
                dr_matmuls(
                    ps[:mw, :cw],
                    lambda j: xv[:, 1, 2 * j : 2 * j + 2, m0 : m0 + mw],
                    lambda k: xv[:, :, k, m0 : m0 + mw],
                    lambda j: wvv[:, 0, 2 * j : 2 * j + 2, c0 : c0 + cw],
                    lambda k: wvv[:, :, k, c0 : c0 + cw],
                )
                nh = cw // HD
                dst = vt[b][m][:mw, ci * nh * (HD + 1) : (ci + 1) * nh * (HD + 1)]
                dst = dst.rearrange("p (h e) -> p h e", h=nh)[:, :, 0:HD]
                src = ps[:mw, :cw].rearrange("p (h e) -> p h e", h=nh)
                nc.vector.tensor_copy(dst, src)

        def emit_qkv_qk(b, ct, ts=None):
            xv = x8v(b)
            wqv = wqk8v()
            for t in ts if ts is not None else (ct, KC + ct):
                for n0, nw in FC:
                    ps = ps_mm.tile([128, 512], F32, name=f"psqk{b}_{t}_{n0}", tag="mm")
                    dr_matmuls(
                        ps[:, :nw],
                        lambda j: wqv[:, 0, 2 * j : 2 * j + 2, t * 128 : (t + 1) * 128],
                        lambda k: wqv[:, :, k, t * 128 : (t + 1) * 128],
                        lambda j: xv[:, 1, 2 * j : 2 * j + 2, n0 : n0 + nw],
                        lambda k: xv[:, :, k, n0 : n0 + nw],
                    )
                    # psum -> fp8 q/k drain; q (+bias) on ACT, k (plain copy)
                    # on DVE to spread the drain across engines
                    dst = (qt8 if t < KC else kt8)[b]
                    tc_ = t if t < KC else t - KC
                    dslice = dst[:, tc_ * N + n0 : tc_ * N + n0 + nw]
                    with nc.allow_low_precision("fp8 q/k for DoubleRow scores"):
                        if t < KC:
                            nc.scalar.activation(
                                dslice, ps[:, :nw], AF.Identity,
                                bias=qkb[:, t : t + 1],
                            )
                        else:
                            nc.vector.tensor_copy(dslice, ps[:, :nw])

        def emit_shuffle(b, g0, ng):
            """DMA partition-fold for head groups g0..g0+ng-1:
            qs8[64*jt+32*jo+p, g*2*NP+i*NP+n] = qt8[64*jo+32*i+p, (2g+jt)*N+n].
            One DMA per (side, jt, i, jo), g as a strided middle dim; every AP
            is a single-partition-dim 3-dim pattern.
            """
            for srcb, dstb in ((qt8[b], qs8[b]), (kt8[b], ks8[b])):
                for jt in range(2):
                    for i in range(2):
                        for jo in range(2):
                            sv = srcb[64 * jo + 32 * i : 64 * jo + 32 * i + 32, :].rearrange(
                                "p (t2 two n) -> p t2 two n", two=2, n=N
                            )[:, g0 : g0 + ng, jt, :]
                            dv = dstb[64 * jt + 32 * jo : 64 * jt + 32 * jo + 32, :].rearrange(
                                "p (t2 two n) -> p t2 two n", two=2, n=NP
                            )[:, g0 : g0 + ng, i, 0:N]
                            nc.sync.dma_start(dv, sv)

        def emit_eb(h, tagsfx):
            eb = ebp.tile([128, 5 * N], BF16, name=f"eb{h}{tagsfx}", tag="eb")
            nc.sync.dma_start(
                eb[:].rearrange("p (m n) -> p m n", m=5),
                eb_d[h].rearrange("(m p) n -> p m n", p=128),
            )
            return eb

        def emit_att_st(h, b, eb, st, ms):
            """S^T DR matmuls + exp + bias-mult for m-tiles `ms` of one head.
            `st` accumulates the est/e1 pair tiles across partial calls."""
            g, j = h // 4, h % 4
            qv = qs8[b][32 * j : 32 * j + 32, g * 2 * NP : (g + 1) * 2 * NP].rearrange(
                "p (i n) -> p i n", i=2
            )
            kv = ks8[b][32 * j : 32 * j + 32, g * 2 * NP : (g + 1) * 2 * NP].rearrange(
                "p (i n) -> p i n", i=2
            )
            for m in ms:
                m0, mw = NT[m]
                if m % 2 == 0:
                    w = 2 * N if m + 1 < len(NT) else N
                    st.append(
                        (estp.tile([128, w], BF16, name=f"est{h}_{b}_{m}", tag="est"),
                         e1p.tile([128, w], BF16, name=f"e1{h}_{b}_{m}", tag="e1"))
                    )
                pcur, pe1 = st[-1]
                co = (m % 2) * N
                ps = ps_st.tile([128, N], F32, name=f"pst{h}_{b}_{m}", tag="st")
                for n0, nw in FB:
                    nc.tensor.matmul(
                        ps[:mw, n0 : n0 + nw],
                        kv[:, :, m0 : m0 + mw],
                        qv[:, :, n0 : n0 + nw],
                        start=True,
                        stop=True,
                        perf_mode=PM.DoubleRow,
                        tile_position=(32 * j, 0),
                    )
                nc.scalar.activation(pe1[:mw, co : co + N], ps[:mw, :], AF.Exp, scale=EXP_SCALE)
                if m % 2 == 1 or m == len(NT) - 1:
                    pw_ = co + N
                    # eb multiply: all-SBUF bf16, so Pool can help; route odd
                    # heads' pair tiles to Pool to offload the busy DVE
                    eng = nc.gpsimd if (h % 2 == 1 and m % 2 == 1) else nc.vector
                    eng.tensor_mul(
                        pcur[:mw, 0:pw_],
                        pe1[:mw, 0:pw_],
                        eb[:mw, (m - pw_ // N + 1) * N : (m + 1) * N],
                    )

        def emit_att_pv(h, b, st):
            ctq = h // 2
            off = (h % 2) * HD
            rr = rowp.tile([1, N], BF16, name=f"rr{h}_{b}", tag="rr")
            bc = bcp.tile([HD, N], BF16, name=f"bc{h}_{b}", tag="bc")
            pvs = []
            for fi, (n0, nw) in enumerate(FB):
                pv = ps_pv.tile([HD + 1, 512], F32, name=f"pv{h}_{b}_{fi}", tag="pv")
                pvs.append(pv)
                for m, (m0, mw) in enumerate(NT):
                    sl = (m % 2) * N + n0
                    nc.tensor.matmul(
                        pv[: HD + 1, :nw],
                        vt[b][m][:mw, h * (HD + 1) : (h + 1) * (HD + 1)],
                        st[m // 2][0][:mw, sl : sl + nw],
                        start=(m == 0),
                        stop=(m == len(NT) - 1),
                    )
                with nc.allow_low_precision("softmax denominator recip in bf16"):
                    nc.vector.reciprocal(rr[0:1, n0 : n0 + nw], pv[HD : HD + 1, :nw])
            nc.gpsimd.partition_broadcast(bc[:, :], rr[0:1, :], channels=HD)
            for fi, (n0, nw) in enumerate(FB):
                nc.vector.tensor_mul(
                    ot[b][ctq][off : off + HD, n0 : n0 + nw],
                    pvs[fi][0:HD, :nw],
                    bc[:, n0 : n0 + nw],
                )

        def emit_proj(b, cot):
            """fin^T[co, n] = pw-block.T-slices @ ot; bias is per-partition
            here; host transposes the (C, N) output back to (N, C)."""
            fin = finp.tile([128, N], BF16, name=f"fin{b}_{cot}", tag="fin")
            for n0, nw in FB:
                ps = ps_mm.tile([128, 512], F32, name=f"psp{b}_{cot}_{n0}", tag="mm")
                for k in range(KC):
                    nc.tensor.matmul(
                        ps[:, :nw],
                        pw[:, k * C + cot * 128 : k * C + (cot + 1) * 128],
                        ot[b][k][:, n0 : n0 + nw],
                        start=(k == 0),
                        stop=(k == KC - 1),
                    )
                # alternate ACT/DVE so neither serializes the tail
                if cot % 2 == 0:
                    nc.scalar.activation(
                        fin[:, n0 : n0 + nw],
                        ps[:, :nw],
                        AF.Identity,
                        bias=pbc[:, cot : cot + 1],
                    )
                else:
                    nc.vector.tensor_scalar_add(
                        fin[:, n0 : n0 + nw], ps[:, :nw], pbc[:, cot : cot + 1]
                    )
            nc.sync.dma_start(out_d[b, cot * 128 : (cot + 1) * 128, :], fin[:, :])

        # ---- emission schedule ----
        # software-pipelined attention: head h+1's S^T (m0-m3) is emitted
        # before head h's PV so PE never sits behind the exp/mult chain;
        # QKV/proj groups fill the remaining per-head PE slack.
        def att_pass(b, tagsfx, fillers):
            stt = {}

            def lead(h):
                eb = emit_eb(h, tagsfx)
                stt[h] = []
                emit_att_st(h, b, eb, stt[h], range(0, 4))
                return eb

            eb = lead(0)
            ebs = {0: eb}
            emit_att_st(0, b, ebs[0], stt[0], [4])
            for h in range(H):
                if h + 1 < H:
                    ebs[h + 1] = lead(h + 1)
                emit_att_pv(h, b, stt.pop(h))
                if h + 1 < H:
                    emit_att_st(h + 1, b, ebs[h + 1], stt[h + 1], [4])
                for f in fillers.get(h, ()):
                    f()

        # pass 1: b0 attention; fillers: rest of b0 qkv, all of b1 qkv
        emit_qkv_qk(0, 0)
        emit_qkv_qk(0, 1)
        emit_shuffle(0, 0, 1)
        emit_qkv_v(0, 0)

        def dma_pw():
            # proj weights aren't consumed until pass 2; issue them here so
            # they never queue ahead of eb tiles on SP
            nc.sync.dma_start(
                pw[:].rearrange("p (k c) -> p k c", k=KC),
                pw_d[:].rearrange("(k p) c -> p k c", p=128),
            )
            nc.sync.dma_start(pbc[:], pbc_d[:])

        f1 = {
            0: (lambda: emit_qkv_v(0, 1), lambda: emit_qkv_qk(0, 2),
                lambda: emit_qkv_qk(0, 3)),
            1: (lambda: emit_qkv_v(0, 2), lambda: emit_qkv_qk(0, 4),
                lambda: emit_qkv_qk(0, 5)),
            2: (lambda: emit_qkv_v(0, 3), lambda: emit_shuffle(0, 1, 2)),
            3: (lambda: emit_qkv_v(0, 4),),
            4: (lambda: emit_qkv_v(1, 0),),
            5: (lambda: emit_qkv_v(1, 1),),
            6: (lambda: emit_qkv_v(1, 2), dma_pw),
            7: (lambda: emit_qkv_v(1, 3),),
            8: (lambda: emit_qkv_v(1, 4), lambda: emit_qkv_qk(1, 0)),
            9: (lambda: emit_qkv_qk(1, 1), lambda: emit_qkv_qk(1, 2)),
            10: (lambda: emit_qkv_qk(1, 3), lambda: emit_qkv_qk(1, 4)),
            11: (lambda: emit_qkv_qk(1, 5), lambda: emit_shuffle(1, 0, 3)),
        }
        att_pass(0, "a", f1)
        # pass 2: b1 attention; fillers: b0 proj; b1 proj is the tail
        f2 = {0: (lambda: emit_proj(0, 0),)}
        for h in (1, 3, 5, 7, 9):
            f2[h] = (lambda c=(h + 1) // 2: emit_proj(0, c),)
        att_pass(1, "b", f2)
        for cot in range(KC):
            emit_proj(1, cot)

    nc.compile()
    return nc


_NC = None


def get_compiled():
    global _NC
    if _NC is None:
        _NC = build_graph()
    return _NC


def fp8_hilo(a):
    """Split fp32 array into (lo, hi) fp8e4m3 with hi + lo ~= a."""
    hi = a.astype(NPFP8)
    lo = (a - hi.astype(np.float32)).astype(NPFP8)
    return lo, hi


def prep_in_maps(x, rel_pos_bias, qkv_weight, q_bias, v_bias, proj_weight, proj_bias):
    x = np.asarray(x, np.float32)
    rel_pos_bias = np.asarray(rel_pos_bias, np.float32)
    qkv_weight = np.asarray(qkv_weight, np.float32)
    q_bias = np.asarray(q_bias, np.float32)
    v_bias = np.asarray(v_bias, np.float32)
    proj_weight = np.asarray(proj_weight, np.float32)
    proj_bias = np.asarray(proj_bias, np.float32)

    # x8: (B, 128, 2(lo,hi), KC, N)
    xT = np.ascontiguousarray(x.transpose(0, 2, 1))  # (B, C, N)
    xk = xT.reshape(B, KC, 128, N).transpose(0, 2, 1, 3)  # (B, 128, KC, N)
    x_lo, x_hi = fp8_hilo(xk)
    x8 = np.zeros((B, 128, 2, KC, NP), NPFP8)
    x8[..., :N] = np.stack([x_lo, x_hi], axis=2)

    # w8: (128, 2(hi,lo), KC, chan), pre-scaled by WS
    def w8_of(wT, chan):  # wT (C, chan)
        wk = (WS * wT).reshape(KC, 128, chan).transpose(1, 0, 2)  # (128, KC, chan)
        lo, hi = fp8_hilo(wk)
        return np.ascontiguousarray(np.stack([hi, lo], axis=1))  # (128,2,KC,chan)

    wqk8 = w8_of(qkv_weight[: 2 * C].T, 2 * C)
    wv8 = w8_of(qkv_weight[2 * C :].T, C)
    pwT = np.ascontiguousarray(proj_weight.T / WS).astype(NPBF16)  # (C, C)
    ebT = np.zeros((H, 640, N), NPBF16)
    ebT[:, :N] = np.exp(rel_pos_bias.transpose(0, 2, 1).astype(np.float64)).astype(NPBF16)

    qkb = np.zeros((128, 2 * KC), np.float32)
    for t in range(KC):
        qkb[:, t] = WS * q_bias[t * 128 : (t + 1) * 128]

    pbe = (proj_bias + v_bias @ proj_weight.T).astype(np.float32)  # (C,)
    pbc = np.ascontiguousarray(pbe.reshape(KC, 128).T)  # [p, cot] = pbe[cot*128+p]

    shared = {"wqk8": wqk8, "wv8": wv8, "pwT": pwT, "ebT": ebT, "qkb": qkb, "pbc": pbc}
    in_maps = []
    for i in range(NCORES):
        m = dict(shared)
        m["x8"] = np.ascontiguousarray(x8[i * BL : (i + 1) * BL])
        in_maps.append(m)
    return in_maps


def run(inputs, trace=False, **kw):
    nc = get_compiled()
    in_maps = prep_in_maps(**inputs)
    res = run_bass_kernel_spmd(nc, in_maps, core_ids=list(range(NCORES)), trace=trace, **kw)
    outT = np.concatenate([np.asarray(r["out"]) for r in res.results], axis=0)  # (B, C, N)
    out = np.ascontiguousarray(outT.transpose(0, 2, 1)).astype(np.float32)
    return out, res


def kernel(**inputs):
    out, _ = run(inputs, trace=False)
    return out


# revision 18
# speedup vs baseline: 1.0336x; 1.0236x over previous
"""BEiT-style attention (B=16, N=577, C=768, H=12) on 8 TRN2 NeuronCores.

Strategy: pure data-parallel over batch (2 batches/core, no collectives).
Per-core kernel computes attention in a transposed-score layout (S^T with
softmax axis on partitions) which needs zero on-device transposes:

  qT,kT  [d, n] = Wqk8 @ x8^T        fp8 DoubleRow hi/lo (4.5 units vs 6)
  q8,k8  [32, 2, n]                  DMA partition-fold, 4 heads/tile
  S^T    [m, n] = k8.T-slices @ q8   fp8 DoubleRow (0.5 cycles/row)
  expS^T [m, n] = exp(sc*S^T) * exp(relposT)   (scale folded into ACT)
  outT   [d, n] = [v | 1].T @ expS^T (row 64 = softmax denominator; bf16)
  out    [n,co] = outT_norm.T-slices @ Wp^T + bias  (bf16 out, host upcast)

fp8 trick: weights pre-scaled by 32 on host (std 0.02 is fp8-denormal
territory); 1/32^2 folded into the exp scale, 1/32 into proj weights.
x and W are split hi/lo fp8 so DoubleRow matmuls carry ~2^-8 relative
error (better than bf16) at 0.5 cycles/row.
"""

import os
import sys
from contextlib import ExitStack

import numpy as np

sys.path.insert(0, "/opt/trn_rl_repo")

# the kernel executes through jax/PJRT on the axon-tunneled NeuronCores; a
# JAX_PLATFORMS=cpu pin (useful for pure-reference runs) would hide them
if os.environ.get("JAX_PLATFORMS", "") == "cpu":
    os.environ.pop("JAX_PLATFORMS", None)

import ml_dtypes  # noqa: E402

from concourse import bacc, mybir  # noqa: E402
import concourse.bass as bass  # noqa: E402
import concourse.tile as tile  # noqa: E402
from concourse.bass_utils import run_bass_kernel_spmd  # noqa: E402

BF16 = mybir.dt.bfloat16
F32 = mybir.dt.float32
FP8 = mybir.dt.float8e4
NPBF16 = ml_dtypes.bfloat16
NPFP8 = ml_dtypes.float8_e4m3
AF = mybir.ActivationFunctionType
PM = mybir.MatmulPerfMode

B, N, C = 16, 577, 768
H, HD = 12, 64
NCORES = 8
BL = B // NCORES  # local batches per core
KC = C // 128  # contraction tiles over channels
SCALE = HD ** -0.5
WS = 32.0  # host weight pre-scale (exact power of 2)
EXP_SCALE = SCALE / (WS * WS)

# token-dim partition tiles (offset, width)
NP = 592  # padded token-dim slot: DoubleRow pair strides must be %16==0
NT = [(0, 128), (128, 128), (256, 128), (384, 128), (512, 65)]
# token-dim free chunks (halves of 577, each fits one PSUM bank / <=512 mm)
FC = [(0, 289), (289, 288)]
# chunks aligned to PSUM bank boundary for fused two-bank [.,577] psum tiles
FB = [(0, 512), (512, 65)]
# channel free chunks for 768-wide outputs
CC = [(0, 384), (384, 384)]


def build_graph():
    nc = bacc.Bacc("TRN2", target_bir_lowering=False, debug=False, num_devices=NCORES)

    # x8: [BL, 128, 2(hl: lo,hi), KC, N]; w8: [128, 2(hl: hi,lo), KC, chan]
    x8_d = nc.dram_tensor("x8", (BL, 128, 2, KC, NP), FP8, kind="ExternalInput").ap()
    wqk_d = nc.dram_tensor("wqk8", (128, 2, KC, 2 * C), FP8, kind="ExternalInput").ap()
    wv_d = nc.dram_tensor("wv8", (128, 2, KC, C), FP8, kind="ExternalInput").ap()
    pw_d = nc.dram_tensor("pwT", (C, C), BF16, kind="ExternalInput").ap()
    eb_d = nc.dram_tensor("ebT", (H, 640, N), BF16, kind="ExternalInput").ap()
    qkb_d = nc.dram_tensor("qkb", (128, 2 * KC), F32, kind="ExternalInput").ap()
    pbc_d = nc.dram_tensor("pbc", (128, KC), F32, kind="ExternalInput").ap()
    out_d = nc.dram_tensor("out", (BL, C, N), BF16, kind="ExternalOutput").ap()

    with tile.TileContext(nc) as tc, ExitStack() as ctx:
        res = ctx.enter_context(tc.tile_pool(name="res", bufs=1))
        ebp = ctx.enter_context(tc.tile_pool(name="ebp", bufs=4))
        e1p = ctx.enter_context(tc.tile_pool(name="e1p", bufs=6))
        estp = ctx.enter_context(tc.tile_pool(name="estp", bufs=8))
        rowp = ctx.enter_context(tc.tile_pool(name="rowp", bufs=6))
        bcp = ctx.enter_context(tc.tile_pool(name="bcp", bufs=6))
        finp = ctx.enter_context(tc.tile_pool(name="finp", bufs=4))
        ps_mm = ctx.enter_context(
            tc.tile_pool(name="ps_mm", bufs=2, space=bass.MemorySpace.PSUM)
        )
        ps_st = ctx.enter_context(
            tc.tile_pool(name="ps_st", bufs=2, space=bass.MemorySpace.PSUM)
        )
        ps_pv = ctx.enter_context(
            tc.tile_pool(name="ps_pv", bufs=2, space=bass.MemorySpace.PSUM)
        )

        # ---- resident tiles ----
        wqk = res.tile([128, 2 * KC * 2 * C], FP8, name="wqk", tag="wqk")
        wv = res.tile([128, 2 * KC * C], FP8, name="wv", tag="wv")
        pw = res.tile([128, KC * C], BF16, name="pw", tag="pw")
        xt = [res.tile([128, 2 * KC * NP], FP8, name=f"xt{b}", tag=f"xt{b}") for b in range(BL)]
        # pre-shuffle q/k (fp8, d on partitions): col t*N.. = chan block t
        qt8 = [res.tile([128, KC * N], FP8, name=f"qt8{b}", tag=f"qt8{b}") for b in range(BL)]
        kt8 = [res.tile([128, KC * N], FP8, name=f"kt8{b}", tag=f"kt8{b}") for b in range(BL)]
        # post-shuffle packed (4 heads x 32 partitions, halves in free dim)
        qs8 = [res.tile([128, 3 * 2 * NP], FP8, name=f"qs8{b}", tag=f"qs8{b}") for b in range(BL)]
        ks8 = [res.tile([128, 3 * 2 * NP], FP8, name=f"ks8{b}", tag=f"ks8{b}") for b in range(BL)]
        vt = [
            [res.tile([128, H * (HD + 1)], BF16, name=f"vt{b}_{m}", tag=f"vt{b}_{m}") for m in range(len(NT))]
            for b in range(BL)
        ]
        ot = [
            [res.tile([128, N], BF16, name=f"ot{b}_{k}", tag=f"ot{b}_{k}") for k in range(KC)]
            for b in range(BL)
        ]
        qkb = res.tile([128, 2 * KC], F32, name="qkb_s", tag="qkb_s")
        pbc = res.tile([128, KC], F32, name="pbc_s", tag="pbc_s")

        def x8v(b):  # [128, hl, kc, NP]
            return xt[b][:].rearrange("p (a k n) -> p a k n", a=2, k=KC)

        def wqk8v():  # [128, hl, kc, 2C]
            return wqk[:].rearrange("p (a k c) -> p a k c", a=2, k=KC)

        def wv8v():  # [128, hl, kc, C]
            return wv[:].rearrange("p (a k c) -> p a k c", a=2, k=KC)

        # ---- input DMA: q/k weights + x(hi) first (gate the first matmuls) ----
        # x8 layout hl: 0=lo 1=hi ; w8 layout hl: 0=hi 1=lo
        def dma_x(b, hl, k0, kn, eng=None):
            (eng or nc.sync).dma_start(
                x8v(b)[:, hl, k0 : k0 + kn, :],
                x8_d[b, :, hl, k0 : k0 + kn, :],
            )

        def dma_w(dst_v, src_d, hl, k0, kn, eng=None):
            (eng or nc.sync).dma_start(
                dst_v[:, hl, k0 : k0 + kn, :],
                src_d[:, hl, k0 : k0 + kn, :],
            )

        dma_w(wqk8v(), wqk_d, 0, 0, 2)
        dma_x(0, 1, 0, 2)
        dma_w(wqk8v(), wqk_d, 1, 0, 2)
        dma_x(0, 0, 0, 2)
        nc.sync.dma_start(qkb[:], qkb_d[:])
        for k0 in (2, 4):
            dma_w(wqk8v(), wqk_d, 0, k0, 2)
            dma_x(0, 1, k0, 2)
            dma_w(wqk8v(), wqk_d, 1, k0, 2)
            dma_x(0, 0, k0, 2)
        for hl in (0, 1):
            dma_w(wv8v(), wv_d, hl, 0, KC)
        for hl in (1, 0):
            dma_x(1, hl, 0, KC)

        # ones column (index HD) interleaved per head in the v tiles
        for b in range(BL):
            for m in range(len(NT)):
                vints = vt[b][m][:].rearrange("p (h e) -> p h e", h=H)
                nc.vector.memset(vints[:, :, HD : HD + 1], 1.0)

        def dr_matmuls(ps_slice, lhs_hi_pair, lhs_cross, rhs_hi_pair, rhs_cross):
            """9 DoubleRow matmuls: 3 hi-hi (kc pairs) + 6 cross (per kc)."""
            first = True
            for j in range(KC // 2):
                nc.tensor.matmul(
                    ps_slice, lhs_hi_pair(j), rhs_hi_pair(j),
                    start=first, stop=False, perf_mode=PM.DoubleRow,
                )
                first = False
            for k in range(KC):
                nc.tensor.matmul(
                    ps_slice, lhs_cross(k), rhs_cross(k),
                    start=False, stop=(k == KC - 1), perf_mode=PM.DoubleRow,
                )

        def emit_qkv_v(b, m, cis=(0, 1)):
            m0, mw = NT[m]
            xv = x8v(b)
            wvv = wv8v()
            for ci in cis:
                c0, cw = CC[ci]
                ps = ps_mm.tile([128, 512], F32, name=f"psv{b}_{m}_{ci}", tag="mm")
                dr_matmuls(
                    ps[:mw, :cw],
                    lambda j: xv[:, 1, 2 * j : 2 * j + 2, m0 : m0 + mw],
                    lambda k: xv[:, :, k, m0 : m0 + mw],
                    lambda j: wvv[:, 0, 2 * j : 2 * j + 2, c0 : c0 + cw],
                    lambda k: wvv[:, :, k, c0 : c0 + cw],
                )
                nh = cw // HD
                dst = vt[b][m][:mw, ci * nh * (HD + 1) : (ci + 1) * nh * (HD + 1)]
                dst = dst.rearrange("p (h e) -> p h e", h=nh)[:, :, 0:HD]
                src = ps[:mw, :cw].rearrange("p (h e) -> p h e", h=nh)
                nc.vector.tensor_copy(dst, src)

        def emit_qkv_qk(b, t, cis=(0, 1)):
            xv = x8v(b)
            wqv = wqk8v()
            if True:
                for n0, nw in (FC[ci] for ci in cis):
                    ps = ps_mm.tile([128, 512], F32, name=f"psqk{b}_{t}_{n0}", tag="mm")
                    dr_matmuls(
                        ps[:, :nw],
                        lambda j: wqv[:, 0, 2 * j : 2 * j + 2, t * 128 : (t + 1) * 128],
                        lambda k: wqv[:, :, k, t * 128 : (t + 1) * 128],
                        lambda j: xv[:, 1, 2 * j : 2 * j + 2, n0 : n0 + nw],
                        lambda k: xv[:, :, k, n0 : n0 + nw],
                    )
                    # psum -> fp8 q/k drain; q (+bias) on ACT, k (plain copy)
                    # on DVE to spread the drain across engines
                    dst = (qt8 if t < KC else kt8)[b]
                    tc_ = t if t < KC else t - KC
                    dslice = dst[:, tc_ * N + n0 : tc_ * N + n0 + nw]
                    with nc.allow_low_precision("fp8 q/k for DoubleRow scores"):
                        if t < KC:
                            nc.scalar.activation(
                                dslice, ps[:, :nw], AF.Identity,
                                bias=qkb[:, t : t + 1],
                            )
                        else:
                            nc.vector.tensor_copy(dslice, ps[:, :nw])

        def emit_shuffle(b, g0, ng):
            """DMA partition-fold for head groups g0..g0+ng-1:
            qs8[64*jt+32*jo+p, g*2*NP+i*NP+n] = qt8[64*jo+32*i+p, (2g+jt)*N+n].
            One DMA per (side, jt, i, jo), g as a strided middle dim; every AP
            is a single-partition-dim 3-dim pattern.
            """
            for srcb, dstb in ((qt8[b], qs8[b]), (kt8[b], ks8[b])):
                for jt in range(2):
                    for i in range(2):
                        for jo in range(2):
                            sv = srcb[64 * jo + 32 * i : 64 * jo + 32 * i + 32, :].rearrange(
                                "p (t2 two n) -> p t2 two n", two=2, n=N
                            )[:, g0 : g0 + ng, jt, :]
                            dv = dstb[64 * jt + 32 * jo : 64 * jt + 32 * jo + 32, :].rearrange(
                                "p (t2 two n) -> p t2 two n", two=2, n=NP
                            )[:, g0 : g0 + ng, i, 0:N]
                            nc.sync.dma_start(dv, sv)

        def emit_eb(h, tagsfx):
            eb = ebp.tile([128, 5 * N], BF16, name=f"eb{h}{tagsfx}", tag="eb")
            nc.sync.dma_start(
                eb[:].rearrange("p (m n) -> p m n", m=5),
                eb_d[h].rearrange("(m p) n -> p m n", p=128),
            )
            return eb

        ebc = [0]

        def emit_att_st(h, b, eb, st, ms):
            """S^T DR matmuls + exp + bias-mult for m-tiles `ms` of one head.
            `st` accumulates the est/e1 pair tiles across partial calls."""
            g, j = h // 4, h % 4
            qv = qs8[b][32 * j : 32 * j + 32, g * 2 * NP : (g + 1) * 2 * NP].rearrange(
                "p (i n) -> p i n", i=2
            )
            kv = ks8[b][32 * j : 32 * j + 32, g * 2 * NP : (g + 1) * 2 * NP].rearrange(
                "p (i n) -> p i n", i=2
            )
            for m in ms:
                m0, mw = NT[m]
                if m % 2 == 0:
                    w = 2 * N if m + 1 < len(NT) else N
                    st.append(
                        (estp.tile([128, w], BF16, name=f"est{h}_{b}_{m}", tag="est"),
                         e1p.tile([128, w], BF16, name=f"e1{h}_{b}_{m}", tag="e1"))
                    )
                pcur, pe1 = st[-1]
                co = (m % 2) * N
                ps = ps_st.tile([128, N], F32, name=f"pst{h}_{b}_{m}", tag="st")
                for n0, nw in FB:
                    nc.tensor.matmul(
                        ps[:mw, n0 : n0 + nw],
                        kv[:, :, m0 : m0 + mw],
                        qv[:, :, n0 : n0 + nw],
                        start=True,
                        stop=True,
                        perf_mode=PM.DoubleRow,
                        tile_position=(32 * j, 0),
                    )
                nc.scalar.activation(pe1[:mw, co : co + N], ps[:mw, :], AF.Exp, scale=EXP_SCALE)
                if m % 2 == 1 or m == len(NT) - 1:
                    pw_ = co + N
                    # eb multiply: all-SBUF bf16, so Pool can help; alternate
                    # full-pair tiles between Pool and DVE to split the load
                    if m % 2 == 1:
                        ebc[0] += 1
                        eng = nc.gpsimd if ebc[0] % 2 == 0 else nc.vector
                    else:
                        eng = nc.vector
                    eng.tensor_mul(
                        pcur[:mw, 0:pw_],
                        pe1[:mw, 0:pw_],
                        eb[:mw, (m - pw_ // N + 1) * N : (m + 1) * N],
                    )

        def emit_att_pv(h, b, st):
            ctq = h // 2
            off = (h % 2) * HD
            rr = rowp.tile([1, N], BF16, name=f"rr{h}_{b}", tag="rr")
            bc = bcp.tile([HD, N], BF16, name=f"bc{h}_{b}", tag="bc")
            pvs = []
            for fi, (n0, nw) in enumerate(FB):
                pv = ps_pv.tile([HD + 1, 512], F32, name=f"pv{h}_{b}_{fi}", tag="pv")
                pvs.append(pv)
                for m, (m0, mw) in enumerate(NT):
                    sl = (m % 2) * N + n0
                    nc.tensor.matmul(
                        pv[: HD + 1, :nw],
                        vt[b][m][:mw, h * (HD + 1) : (h + 1) * (HD + 1)],
                        st[m // 2][0][:mw, sl : sl + nw],
                        start=(m == 0),
                        stop=(m == len(NT) - 1),
                    )
                with nc.allow_low_precision("softmax denominator recip in bf16"):
                    nc.vector.reciprocal(rr[0:1, n0 : n0 + nw], pv[HD : HD + 1, :nw])
            nc.gpsimd.partition_broadcast(bc[:, :], rr[0:1, :], channels=HD)
            for fi, (n0, nw) in enumerate(FB):
                nc.vector.tensor_mul(
                    ot[b][ctq][off : off + HD, n0 : n0 + nw],
                    pvs[fi][0:HD, :nw],
                    bc[:, n0 : n0 + nw],
                )

        def emit_proj(b, cot, fis=(0, 1)):
            """fin^T[co, n] = pw-block.T-slices @ ot + bias; out DMA'd bf16,
            host transposes/upcasts. Bias drain alternates ACT/DVE."""
            fin = finp.tile([128, N], BF16, name=f"fin{b}_{cot}", tag="fin")
            for fi in fis:
                n0, nw = FB[fi]
                ps = ps_mm.tile([128, 512], F32, name=f"psp{b}_{cot}_{fi}", tag="mm")
                for k in range(KC):
                    nc.tensor.matmul(
                        ps[:, :nw],
                        pw[:, k * C + cot * 128 : k * C + (cot + 1) * 128],
                        ot[b][k][:, n0 : n0 + nw],
                        start=(k == 0),
                        stop=(k == KC - 1),
                    )
                if cot % 2 == 0:
                    nc.scalar.activation(
                        fin[:, n0 : n0 + nw], ps[:, :nw], AF.Identity,
                        bias=pbc[:, cot : cot + 1],
                    )
                else:
                    nc.vector.tensor_scalar_add(
                        fin[:, n0 : n0 + nw], ps[:, :nw], pbc[:, cot : cot + 1]
                    )
                nc.sync.dma_start(
                    out_d[b, cot * 128 : (cot + 1) * 128, n0 : n0 + nw],
                    fin[:, n0 : n0 + nw],
                )

        # ---- emission schedule ----
        # Single interleaved stream: b0/b1 attention heads alternate through
        # the span (ACT is the pacer); head h+1's S^T (m0-m3) is emitted
        # before head h's PV; single-PSUM-group QKV/proj units are popped
        # from a queue at two points per head slot so ps_mm rotation never
        # stalls PE behind a drain.
        SLOTS = (
            [(0, h) for h in range(6)]
            + [p for h in range(6) for p in ((1, h), (0, h + 6))]
            + [(1, h) for h in range(6, 12)]
        )
        fq = []  # (cost_us, thunk, mark)
        marks = set()

        def pop_one():
            cost, f, mark = fq.pop(0)
            f()
            if mark is not None:
                marks.add(mark)
            return cost

        def pop_units(budget):
            while fq and budget > 0:
                budget -= pop_one()

        def flush_until(mark):
            while fq and mark not in marks:
                pop_one()

        qcost, vcost, pcost = 0.55, 0.75, 0.8
        for t in (0, 6, 1, 7):
            emit_qkv_qk(0, t)
        emit_shuffle(0, 0, 1)
        for t in (2, 8, 3, 9):
            for ci in (0, 1):
                fq.append((qcost, lambda t=t, ci=ci: emit_qkv_qk(0, t, (ci,)), None))
        fq.append((0.0, lambda: emit_shuffle(0, 1, 1), ("shuf", 0, 1)))
        for t in (4, 10, 5, 11):
            for ci in (0, 1):
                fq.append((qcost, lambda t=t, ci=ci: emit_qkv_qk(0, t, (ci,)), None))
        fq.append((0.0, lambda: emit_shuffle(0, 2, 1), ("shuf", 0, 2)))
        for t in (0, 6, 1, 7):
            for ci in (0, 1):
                fq.append((qcost, lambda t=t, ci=ci: emit_qkv_qk(1, t, (ci,)), None))
        fq.append((0.0, lambda: emit_shuffle(1, 0, 1), ("shuf", 1, 0)))
        for m in range(len(NT)):
            for ci in (0, 1):
                fq.append((vcost, lambda m=m, ci=ci: emit_qkv_v(1, m, (ci,)), None))
        fq.append((0.0, lambda: None, ("v", 1)))
        for t in (2, 8, 3, 9):
            for ci in (0, 1):
                fq.append((qcost, lambda t=t, ci=ci: emit_qkv_qk(1, t, (ci,)), None))
        fq.append((0.0, lambda: emit_shuffle(1, 1, 1), ("shuf", 1, 1)))
        for t in (4, 10, 5, 11):
            for ci in (0, 1):
                fq.append((qcost, lambda t=t, ci=ci: emit_qkv_qk(1, t, (ci,)), None))
        fq.append((0.0, lambda: emit_shuffle(1, 2, 1), ("shuf", 1, 2)))

        def dma_pw():
            nc.sync.dma_start(
                pw[:].rearrange("p (k c) -> p k c", k=KC),
                pw_d[:].rearrange("(k p) c -> p k c", p=128),
            )
            nc.sync.dma_start(pbc[:], pbc_d[:])

        fq.append((0.0, dma_pw, None))

        stt = {}
        ebs = {}

        def lead(s):
            b, h = s
            if b == 1 or h >= 4:
                flush_until(("shuf", b, h // 4))
            ebs[s] = emit_eb(h, "ab"[b])
            stt[s] = []
            emit_att_st(h, b, ebs[s], stt[s], range(0, 4))

        lead(SLOTS[0])
        for ci in (0, 1):
            for m in range(len(NT)):
                emit_qkv_v(0, m, (ci,))
        emit_att_st(0, 0, ebs[SLOTS[0]], stt[SLOTS[0]], [4])
        for si, s in enumerate(SLOTS):
            b, h = s
            if si + 1 < len(SLOTS):
                lead(SLOTS[si + 1])
                pop_units(1.2)
            if b == 1:
                flush_until(("v", 1))
            emit_att_pv(h, b, stt.pop(s))
            if si + 1 < len(SLOTS):
                nb, nh = SLOTS[si + 1]
                emit_att_st(nh, nb, ebs[SLOTS[si + 1]], stt[SLOTS[si + 1]], [4])
            pop_units(1.2 if si < len(SLOTS) - 2 else 99.0)
            if s == (0, 11):
                # b0 fully done: release its proj into the filler queue
                for cot in range(KC):
                    for fi in (0, 1):
                        fq.append(
                            (pcost if fi == 0 else 0.2,
                             lambda cot=cot, fi=fi: emit_proj(0, cot, (fi,)), None)
                        )
        for cot in range(KC):
            emit_proj(1, cot)

    nc.compile()
    return nc


_NC = None


def get_compiled():
    global _NC
    if _NC is None:
        _NC = build_graph()
    return _NC


def fp8_hilo(a):
    """Split fp32 array into (lo, hi) fp8e4m3 with hi + lo ~= a."""
    hi = a.astype(NPFP8)
    lo = (a - hi.astype(np.float32)).astype(NPFP8)
    return lo, hi


def prep_in_maps(x, rel_pos_bias, qkv_weight, q_bias, v_bias, proj_weight, proj_bias):
    x = np.asarray(x, np.float32)
    rel_pos_bias = np.asarray(rel_pos_bias, np.float32)
    qkv_weight = np.asarray(qkv_weight, np.float32)
    q_bias = np.asarray(q_bias, np.float32)
    v_bias = np.asarray(v_bias, np.float32)
    proj_weight = np.asarray(proj_weight, np.float32)
    proj_bias = np.asarray(proj_bias, np.float32)

    # x8: (B, 128, 2(lo,hi), KC, N)
    xT = np.ascontiguousarray(x.transpose(0, 2, 1))  # (B, C, N)
    xk = xT.reshape(B, KC, 128, N).transpose(0, 2, 1, 3)  # (B, 128, KC, N)
    x_lo, x_hi = fp8_hilo(xk)
    x8 = np.zeros((B, 128, 2, KC, NP), NPFP8)
    x8[..., :N] = np.stack([x_lo, x_hi], axis=2)

    # w8: (128, 2(hi,lo), KC, chan), pre-scaled by WS
    def w8_of(wT, chan):  # wT (C, chan)
        wk = (WS * wT).reshape(KC, 128, chan).transpose(1, 0, 2)  # (128, KC, chan)
        lo, hi = fp8_hilo(wk)
        return np.ascontiguousarray(np.stack([hi, lo], axis=1))  # (128,2,KC,chan)

    wqk8 = w8_of(qkv_weight[: 2 * C].T, 2 * C)
    wv8 = w8_of(qkv_weight[2 * C :].T, C)
    pwT = np.ascontiguousarray(proj_weight.T / WS).astype(NPBF16)  # (C, C)
    ebT = np.zeros((H, 640, N), NPBF16)
    ebT[:, :N] = np.exp(rel_pos_bias.transpose(0, 2, 1).astype(np.float64)).astype(NPBF16)

    qkb = np.zeros((128, 2 * KC), np.float32)
    for t in range(KC):
        qkb[:, t] = WS * q_bias[t * 128 : (t + 1) * 128]

    pbe = (proj_bias + v_bias @ proj_weight.T).astype(np.float32)  # (C,)
    pbc = np.ascontiguousarray(pbe.reshape(KC, 128).T)  # [p, cot] = pbe[cot*128+p]

    shared = {"wqk8": wqk8, "wv8": wv8, "pwT": pwT, "ebT": ebT, "qkb": qkb, "pbc": pbc}
    in_maps = []
    for i in range(NCORES):
        m = dict(shared)
        m["x8"] = np.ascontiguousarray(x8[i * BL : (i + 1) * BL])
        in_maps.append(m)
    return in_maps


def run(inputs, trace=False, **kw):
    nc = get_compiled()
    in_maps = prep_in_maps(**inputs)
    res = run_bass_kernel_spmd(nc, in_maps, core_ids=list(range(NCORES)), trace=trace, **kw)
    outT = np.concatenate([np.asarray(r["out"], np.float32) for r in res.results], axis=0)
    out = np.ascontiguousarray(outT.transpose(0, 2, 1))
    return out, res


def kernel(**inputs):
    out, _ = run(inputs, trace=False)
    return out
